# revision 43
# baseline (speedup 1.0000x reference)
"""Causal attention (B=4, S=2048, D=1024, single head) on 8 TRN2 NeuronCores.

Sharding: data-parallel over batch (4 pairs of cores); within each pair
the K/V context is split by interleaved 128-row chunks (core parity p
owns global k-chunks {2j+p}).

Algorithm (v4): scores = q k^T = x (Wq^T Wk) x^T.  The host precomputes
M = Wq^T Wk in fp32 (cast to bf16), so the device projects only
G = x M (the exact cost of the old Q projection) and V = x Wv^T; the
K projection disappears entirely and scores are computed as G x^T
against the x tiles already resident in SBUF.

Own-first gathered layout (v4): each core's gathered q-column order is
[its OWN 8 blocks | its PEER's 8 blocks] (not the canonical rank-indexed
order).  The own half of G^T feeds attention tiles 0/1 straight from
the projection copies -- no exchange dependency at all -- so the PE has
~31us of own-tile work to chew on while the exchange runs.  Only tiles
2/3 (peer half) wait on it.

The peer half arrives via pairwise AllGather + a negate-and-accumulate
trick: the AllGather output is rank-indexed (parity-dependent
addressing, not SPMD-expressible), so the peer region is seeded with
-G_own (scalar activation, scale=-1) and BOTH rank rows are then
accum-added into it by gpsimd SWDGE DMAs, leaving exactly G_peer.
The combine must be DMA-only: the Tile scheduler orders engine queues
by its own cost model, and any mesh-gated compute op placed on a hot
queue (vector/scalar) blocks that queue's copies for ~10us.
(A pairwise ReduceScatter with parity-masked slots gives the peer half
directly, but RS meshes run ~2x slower than AllGather here, ~38us vs
~18us, which put the piece-1 exchange back on the critical path.)

Causal structure in own-first order is parity-symmetric for the own
half (diagonal blocks masked by one shared lower-triangle) and handled
for the peer half by a per-parity all-ones/all-zeros boundary block;
both live in a tiny [128, 2, 128] mask input.  Block-skip structure
(off/njs) is identical to the canonical layout.

Schedule notes:
- All collectives serialize behind the framework's kernel entry barrier
  (~40us) plus ~12us first-trigger latency, so mesh 0 cannot begin
  before ~52us regardless of staging; the own-first layout gives the
  exchange ~20us of slack instead of sitting on the critical path.
- Exchange stores ride the sync queue (hardware DGE): gpsimd SWDGE
  stores break the collective trigger handshake (CC core sleeps ~70us).
- All matmuls run in bf16 (fp32 PSUM accumulation); partial outputs are
  written in bf16 (host accumulates in fp32).
"""

import sys

if "/opt/trn_rl_repo" not in sys.path:
    sys.path.insert(0, "/opt/trn_rl_repo")

import ml_dtypes
import numpy as np

import concourse.bacc as bacc
import concourse.tile as tile
from concourse import mybir
from concourse.bass_utils import run_bass_kernel_spmd

# bass_utils imports antenv.axon_hooks when tracing is requested; the image's
# antenv lacks that module, so provide a no-op fallback rather than crashing.
try:
    import antenv.axon_hooks  # noqa: F401
except ImportError:
    import types as _types

    _ah = _types.ModuleType("antenv.axon_hooks")
    _ah._hook = None
    _ah.set_axon_ntff_profile_hook = lambda h: setattr(_ah, "_hook", h)
    _ah.get_axon_ntff_profile_hook = lambda: _ah._hook
    sys.modules["antenv.axon_hooks"] = _ah

B, S, D = 4, 2048, 1024
NB = S // 128          # 16 q-blocks of 128 per batch
NT = S // 512          # 4 q-tiles of 512
IC = D // 128          # 8 contraction chunks
LC = 8                 # local k-chunks per core (S/2/128)
SCALE = 1.0 / np.sqrt(D)  # 0.03125
NJ_TILE = [4, 8, 4, 8]  # local k-chunks needed per gathered q-tile
NWARM = 4              # PE warm-up matmuls

BF16 = mybir.dt.bfloat16
F32 = mybir.dt.float32

_module_cache = None
last_results = None  # BassKernelResults of the most recent run (for test harness)


def _masked_js(tt):
    """Local chunk indices whose score blocks have a boundary block."""
    return range(4) if tt in (0, 2) else range(4, 8)


def _build_module():
    nc = bacc.Bacc("TRN2", target_bir_lowering=False, debug=False, num_devices=8)
    # All inputs are packed partition-major on the host so every input DMA
    # moves multi-KB contiguous runs on both the DRAM and SBUF side.
    # xT is half-major: [h, p, i, c]; m4 is M = Wq^T Wk packed as 4
    # o-group slabs [g, p, i, 256].
    xT = nc.dram_tensor("xT", [2, 128, IC, 512], BF16, kind="ExternalInput").ap()
    m4 = nc.dram_tensor("m4", [4, 128, IC, 256], BF16, kind="ExternalInput").ap()
    wvT = nc.dram_tensor("wvT", [128, IC, 1024], BF16, kind="ExternalInput").ap()
    # msk2[:, 0, :] = shared lower-triangle (own diagonal blocks);
    # msk2[:, 1, :] = all-(1-p) (peer boundary blocks)
    msk2 = nc.dram_tensor("msk2", [128, 2, 128], BF16, kind="ExternalInput").ap()
    out_p = nc.dram_tensor("out_p", [S, D], BF16, kind="ExternalOutput").ap()
    rs_out = nc.dram_tensor("rs_out", [1, S], F32, kind="ExternalOutput").ap()

    with tile.TileContext(nc) as tc:
        with (
            tc.tile_pool(name="wp", bufs=1) as wp,
            tc.tile_pool(name="xp", bufs=1) as xp,
            tc.tile_pool(name="kqv", bufs=1) as kqv,
            tc.tile_pool(name="mp", bufs=1) as mp,
            tc.tile_pool(name="ptp", bufs=2) as ptp,
            tc.tile_pool(name="stg", bufs=4) as stg,
            tc.tile_pool(name="dr", bufs=1, space="DRAM") as dr,
        ):
            # ---- PE warm-up: garbage matmuls on zeroed tiles, issued
            #      before any input-dependent work so the PE leaves its
            #      cold clock state while input DMAs are in flight ----
            warm_w = mp.tile([128, 128], BF16, tag="warmw", name="warmw")
            warm_x = mp.tile([128, 512], BF16, tag="warmx", name="warmx")
            nc.gpsimd.memset(warm_w, 0.0)
            nc.gpsimd.memset(warm_x, 0.0)
            with tc.tile_pool(name="psw", bufs=1, space="PSUM") as psw:
                wpp = psw.tile([128, 512], F32, tag="warm", bufs=1, name="warmp")
                for _ in range(NWARM):
                    nc.tensor.matmul(wpp, lhsT=warm_w, rhs=warm_x, start=True, stop=True)

            xt_all = xp.tile([128, 2, IC, 512], BF16, tag="xt", name="xt")
            m_all = wp.tile([128, 4, IC, 256], BF16, tag="m4", name="m4")
            wv_all = wp.tile([128, IC, 1024], BF16, tag="wv", name="wv")
            msk_sb = mp.tile([128, 2, 128], BF16, tag="msk", name="msk")

            # Queues are FIFO, so order input pieces by when the PE needs
            # them.  sync carries x h0 (then exchange stores, output DMAs);
            # scalar carries everything else.  The G-st0 matmul schedule
            # consumes every slab's i0-3 first, then i4-7, matching this
            # arrival order.  (Two variants measured worse: merging the m4
            # quarters into strided DMAs scatters DRAM reads, and moving
            # x h0 onto scalar in need-order shifts the exchange-accum
            # chain later without improving the ~3.4us ramp stall.)
            nc.sync.dma_start(xt_all[:, 0, 0:4, :], xT[0, :, 0:4, :])
            nc.sync.dma_start(xt_all[:, 0, 4:8, :], xT[0, :, 4:8, :])
            for g in range(4):
                nc.scalar.dma_start(m_all[:, g, 0:4, :], m4[g, :, 0:4, :])
            for g in range(4):
                nc.scalar.dma_start(m_all[:, g, 4:8, :], m4[g, :, 4:8, :])
            nc.scalar.dma_start(xt_all[:, 1, :, :], xT[1])
            nc.scalar.dma_start(wv_all, wvT)
            nc.scalar.dma_start(msk_sb, msk2)
            ones_sb = mp.tile([128, 1], BF16, tag="ones", name="ones")
            nc.any.memset(ones_sb, 1.0)

            # gathered G^T, own-first: cols [0, 1024) own halves st0|st1,
            # cols [1024, 2048) peer halves st0|st1
            gt_all = kqv.tile([128, IC, S], BF16, tag="gt", name="gt")
            vn_sb = [kqv.tile([128, D], BF16, tag=f"vn{j}", name=f"vn{j}") for j in range(LC)]
            rs_sb = mp.tile([1, S], F32, tag="rs", name="rs")

            # AllGather bounce buffers (store src reads gt_all's own
            # region directly -- no SBUF staging copy needed)
            qhalf = [dr.tile([128, IC * 512], BF16, name=f"qhalf{st}") for st in range(2)]
            qfull = [dr.tile([2 * 128, IC * 512], BF16, name=f"qfull{st}") for st in range(2)]

            def xs(i, h):
                return xt_all[:, h, i, :]

            def proj_iouter(ps1, lhs_slices, rhs_slices, dsts, pname, schedule=None, tago=0):
                # schedule: list of (i_range, o_list) sub-groups; flags stay
                # start=(i==0)/stop=(i==IC-1) so any i order that runs i==0
                # first and i==IC-1 last per psum is legal
                schedule = schedule or [(range(IC), range(len(dsts)))]
                pps = [
                    ps1.tile(
                        [128, 512], F32, tag=f"proj8_{o + tago}", bufs=1, name=f"{pname}{o}"
                    )
                    for o in range(len(dsts))
                ]
                for irng, orng in schedule:
                    for i in irng:
                        for o in orng:
                            nc.tensor.matmul(
                                pps[o],
                                lhsT=lhs_slices(i, o),
                                rhs=rhs_slices(i, o),
                                start=(i == 0),
                                stop=(i == IC - 1),
                            )
                for o, dst in enumerate(dsts):
                    dst(pps[o])

            def g_dst(o, st):
                def f(pp):
                    nc.vector.tensor_copy(gt_all[:, o, 512 * st : 512 * (st + 1)], pp)
                    # seed the peer region with -G_own; the two AllGather
                    # rows are then accum-added on top, leaving exactly
                    # G_peer (for the rank whose row equals own, -G + G
                    # cancels exactly)
                    nc.scalar.activation(
                        gt_all[:, o, 1024 + 512 * st : 1024 + 512 * (st + 1)],
                        pp,
                        mybir.ActivationFunctionType.Copy,
                        scale=-1.0,
                    )
                return f

            def g_own_phase(ps1, st, schedule=None):
                """Project this core's own G half straight into gt_all's
                own region, stage the parity-masked RS slots, store, and
                trigger the ReduceScatter."""
                proj_iouter(
                    ps1,
                    lambda i, o: m_all[:, o // 2, i, 128 * (o % 2) : 128 * (o % 2 + 1)],
                    lambda i, o: xs(i, st),
                    [g_dst(o, st) for o in range(IC)],
                    f"pg{st}",
                    schedule=schedule,
                )
                nc.sync.dma_start(
                    qhalf[st].rearrange("p (o c) -> p o c", o=IC),
                    gt_all[:, :, 512 * st : 512 * (st + 1)],
                )
                nc.gpsimd.collective_compute(
                    kind="AllGather",
                    op=mybir.AluOpType.bypass,
                    replica_groups=[[0, 1], [2, 3], [4, 5], [6, 7]],
                    ins=[qhalf[st]],
                    outs=[qfull[st]],
                )

            def peer_accum(st, r):
                # accum-add rank row r of piece st onto the -G_own seed
                # (gpsimd SWDGE is the only DGE that supports accum ops)
                c0 = 1024 + 512 * st
                nc.gpsimd.dma_start(
                    gt_all[:, :, c0 : c0 + 512],
                    qfull[st][128 * r : 128 * (r + 1), :].rearrange(
                        "p (o c) -> p o c", o=IC
                    ),
                    accum_op=mybir.AluOpType.add,
                )

            with tc.tile_pool(name="ps1", bufs=1, space="PSUM") as ps1:
                g_own_phase(
                    ps1,
                    0,
                    schedule=[
                        (range(0, 4), [2 * g, 2 * g + 1]) for g in range(4)
                    ]
                    + [(range(4, 8), [2 * g, 2 * g + 1]) for g in range(4)],
                )
                g_own_phase(ps1, 1)
                peer_accum(0, 0)
                peer_accum(0, 1)
                peer_accum(1, 0)
                peer_accum(1, 1)

                # V projection: all 8 local chunks, four 4-psum rounds
                # with alternating bank tags -- the last round drains
                # banks 4-7, so the score pool's banks (0-3) are free the
                # moment attention tile 0 is ready to start
                for rnd in range(4):
                    proj_iouter(
                        ps1,
                        lambda i, c, _r=rnd: xt_all[
                            :, _r // 2, i, 128 * (2 * (_r % 2) + c // 2) : 128 * (2 * (_r % 2) + c // 2 + 1)
                        ],
                        lambda i, c: wv_all[:, i, 512 * (c % 2) : 512 * (c % 2 + 1)],
                        [
                            (
                                lambda dst, eng: lambda pp: eng(dst, pp)
                            )(
                                vn_sb[4 * (rnd // 2) + 2 * (rnd % 2) + c // 2][
                                    :, 512 * (c % 2) : 512 * (c % 2 + 1)
                                ],
                                nc.vector.tensor_copy if c % 2 == 0 else nc.scalar.copy,
                            )
                            for c in range(4)
                        ],
                        f"pv{rnd}",
                        # last round must land on the bank set the score
                        # pool does NOT reuse: the observed ~0.9us stall on
                        # tile 0's first score matmul tracks the final
                        # round's copy drain, so flip which set goes last
                        tago=4 * ((rnd + 1) % 2),
                    )

            # ---- phase 2: attention -- tiles 0/1 (own half) have no
            #      exchange dependency; tiles 2/3 (peer half) ride the
            #      ReduceScatter outputs ----
            with tc.tile_pool(name="ps2", bufs=2, space="PSUM") as ps:

                def attention_tile(tt):
                    nj = NJ_TILE[tt]
                    masked = set(_masked_js(tt))
                    bsel = 0 if tt < 2 else 1  # triangle vs parity block
                    pt_tiles = []
                    offs = []
                    for j in range(nj):
                        # blocks before the boundary block are fully masked
                        # out -- skip computing them entirely
                        off = 128 * (j % 4) if j in masked else 0
                        offs.append(off)
                        sp = ps.tile([128, 512], F32, tag="score", bufs=4, name="score")
                        for i in range(IC):
                            nc.tensor.matmul(
                                sp[:, off:512],
                                lhsT=xt_all[:, j // 4, i, 128 * (j % 4) : 128 * (j % 4 + 1)],
                                rhs=gt_all[:, i, 512 * tt + off : 512 * (tt + 1)],
                                start=(i == 0),
                                stop=(i == IC - 1),
                            )
                        pt = ptp.tile([128, 512], BF16, tag=f"pt{j}", name=f"pt{j}")
                        nc.scalar.activation(
                            pt[:, off:512],
                            sp[:, off:512],
                            mybir.ActivationFunctionType.Exp,
                            scale=SCALE,
                        )
                        if j in masked:
                            # only the 128-wide boundary block needs the
                            # mask; everything past it is fully visible
                            nc.vector.tensor_mul(
                                pt[:, off : off + 128],
                                pt[:, off : off + 128],
                                msk_sb[:, bsel, :],
                            )
                        pt_tiles.append(pt)

                    # partial softmax denominators: ones^T @ pt over j.
                    # (Folding these into the attn@V loop as 1-column
                    # matmuls costs more than it saves: the 3ns matmuls
                    # leave the next group's LDWEIGHTS exposed, ~94ns x 72.)
                    rsp = ps.tile([1, 512], F32, tag="rsp", bufs=1, name="rsp")
                    for j in range(nj):
                        nc.tensor.matmul(
                            rsp[:, offs[j] : 512],
                            lhsT=ones_sb,
                            rhs=pt_tiles[j][:, offs[j] : 512],
                            start=(j == 0),
                            stop=(j == nj - 1),
                        )
                    nc.vector.tensor_copy(rs_sb[:, 512 * tt : 512 * (tt + 1)], rsp)
                    nc.sync.dma_start(
                        rs_out[:, 512 * tt : 512 * (tt + 1)],
                        rs_sb[:, 512 * tt : 512 * (tt + 1)],
                    )

                    for qq in (3, 2, 1, 0):
                        qbg = 4 * tt + qq        # gathered q-block index
                        njs = (qbg % 8) + 1      # causal chunk count
                        ost = stg.tile([128, D], BF16, tag="ost", name="ost")
                        ap0 = ps.tile([128, 512], F32, tag="attn", bufs=3, name="attn")
                        ap1 = ps.tile([128, 512], F32, tag="attn", bufs=3, name="attn")
                        for j in range(njs):
                            lhs = pt_tiles[j][:, 128 * qq : 128 * (qq + 1)]
                            st_, sp_ = (j == 0), (j == njs - 1)
                            nc.tensor.matmul(
                                ap0, lhsT=lhs, rhs=vn_sb[j][:, 0:512], start=st_, stop=sp_
                            )
                            nc.tensor.matmul(
                                ap1, lhsT=lhs, rhs=vn_sb[j][:, 512:1024], start=st_, stop=sp_
                            )
                        nc.vector.tensor_copy(ost[:, 0:512], ap0)
                        nc.sync.dma_start(
                            out_p[128 * qbg : 128 * (qbg + 1), 0:512], ost[:, 0:512]
                        )
                        nc.scalar.copy(ost[:, 512:1024], ap1)
                        nc.sync.dma_start(
                            out_p[128 * qbg : 128 * (qbg + 1), 512:1024],
                            ost[:, 512:1024],
                        )

                attention_tile(0)
                attention_tile(1)
                attention_tile(2)
                attention_tile(3)

    nc.compile()
    return nc


def _get_module():
    global _module_cache
    if _module_cache is None:
        _module_cache = _build_module()
    return _module_cache


def _glob_map(p):
    """Global q row for each own-first gathered position, parity p."""
    pos = np.arange(S)
    blk = pos // 128
    m = blk % 8
    g = np.where(blk < 8, 2 * m + p, 2 * m + (1 - p))
    return 128 * g + pos % 128


def kernel(x, Wq, Wk, Wv, _trace=False):
    global last_results
    nc = _get_module()

    bf = ml_dtypes.bfloat16

    # M = Wq^T Wk in fp32 (scores = x M x^T), packed as 4 o-group slabs
    # [g, p, i, 256], partition-major
    M = Wq.T.astype(np.float32) @ Wk.astype(np.float32)
    m4 = np.ascontiguousarray(
        M.astype(bf).reshape(IC, 128, 4, 256).transpose(2, 1, 0, 3)
    )
    wvT = np.ascontiguousarray(Wv.T.astype(bf).reshape(IC, 128, D).transpose(1, 0, 2))

    # masks: [:, 0, :] lower-triangle (shared), [:, 1, :] all-(1-p)
    k_i = np.arange(128)[:, None]
    q_i = np.arange(128)[None, :]
    tri = (q_i >= k_i).astype(np.float32)
    msk2 = [
        np.stack([tri, np.full((128, 128), 1.0 - par, dtype=np.float32)], axis=1).astype(bf)
        for par in range(2)
    ]
    # per-parity column selection: core owns global k-chunks {2j+par}
    own_cols = [
        (128 * (2 * np.arange(LC)[:, None] + par) + np.arange(128)[None, :]).reshape(-1)
        for par in range(2)
    ]

    in_maps = []
    for c in range(8):
        b, par = c // 2, c % 2
        xTb = x[b].T[:, own_cols[par]].astype(bf)  # [D, S//2]
        xpk = np.ascontiguousarray(
            xTb.reshape(IC, 128, 2, 512).transpose(2, 1, 0, 3)
        )
        in_maps.append(
            {
                "xT": xpk,
                "m4": m4,
                "wvT": wvT,
                "msk2": np.ascontiguousarray(msk2[par]),
            }
        )

    kwargs = {}
    if _trace:
        kwargs["trace"] = True
    res = run_bass_kernel_spmd(nc, in_maps, core_ids=list(range(8)), **kwargs)
    last_results = res

    gmap = [_glob_map(0), _glob_map(1)]

    out = np.empty((B, S, D), dtype=np.float32)
    for b in range(B):
        num = np.zeros((S, D), dtype=np.float32)
        den = np.zeros(S, dtype=np.float32)
        for par in range(2):
            r = res.results[2 * b + par]
            num[gmap[par]] += r["out_p"].astype(np.float32)
            den[gmap[par]] += r["rs_out"][0]
        out[b] = num / den[:, None]
    return out


# revision 44
# speedup vs baseline: 1.0115x; 1.0115x over previous
"""Causal attention (B=4, S=2048, D=1024, single head) on 8 TRN2 NeuronCores.

Sharding: data-parallel over batch (4 pairs of cores); within each pair
the K/V context is split by interleaved 128-row chunks (core parity p
owns global k-chunks {2j+p}).

Algorithm (v4): scores = q k^T = x (Wq^T Wk) x^T.  The host precomputes
M = Wq^T Wk in fp32 (cast to bf16), so the device projects only
G = x M (the exact cost of the old Q projection) and V = x Wv^T; the
K projection disappears entirely and scores are computed as G x^T
against the x tiles already resident in SBUF.

Own-first gathered layout (v4): each core's gathered q-column order is
[its OWN 8 blocks | its PEER's 8 blocks] (not the canonical rank-indexed
order).  The own half of G^T feeds attention tiles 0/1 straight from
the projection copies -- no exchange dependency at all -- so the PE has
~31us of own-tile work to chew on while the exchange runs.  Only tiles
2/3 (peer half) wait on it.

The peer half arrives via pairwise AllGather + a negate-and-accumulate
trick: the AllGather output is rank-indexed (parity-dependent
addressing, not SPMD-expressible), so the peer region is seeded with
-G_own (scalar activation, scale=-1) and BOTH rank rows are then
accum-added into it by gpsimd SWDGE DMAs, leaving exactly G_peer.
The combine must be DMA-only: the Tile scheduler orders engine queues
by its own cost model, and any mesh-gated compute op placed on a hot
queue (vector/scalar) blocks that queue's copies for ~10us.
(A pairwise ReduceScatter with parity-masked slots gives the peer half
directly, but RS meshes run ~2x slower than AllGather here, ~38us vs
~18us, which put the piece-1 exchange back on the critical path.)

Causal structure in own-first order is parity-symmetric for the own
half (diagonal blocks masked by one shared lower-triangle) and handled
for the peer half by a per-parity all-ones/all-zeros boundary block;
both live in a tiny [128, 2, 128] mask input.  Block-skip structure
(off/njs) is identical to the canonical layout.

Schedule notes:
- All collectives serialize behind the framework's kernel entry barrier
  (~40us) plus ~12us first-trigger latency, so mesh 0 cannot begin
  before ~52us regardless of staging; the own-first layout gives the
  exchange ~20us of slack instead of sitting on the critical path.
- Exchange stores ride the sync queue (hardware DGE): gpsimd SWDGE
  stores break the collective trigger handshake (CC core sleeps ~70us).
- All matmuls run in bf16 (fp32 PSUM accumulation); partial outputs are
  written in bf16 (host accumulates in fp32).
"""

import sys

if "/opt/trn_rl_repo" not in sys.path:
    sys.path.insert(0, "/opt/trn_rl_repo")

import ml_dtypes
import numpy as np

import concourse.bacc as bacc
import concourse.tile as tile
from concourse import mybir
from concourse.bass_utils import run_bass_kernel_spmd

# bass_utils imports antenv.axon_hooks when tracing is requested; the image's
# antenv lacks that module, so provide a no-op fallback rather than crashing.
try:
    import antenv.axon_hooks  # noqa: F401
except ImportError:
    import types as _types

    _ah = _types.ModuleType("antenv.axon_hooks")
    _ah._hook = None
    _ah.set_axon_ntff_profile_hook = lambda h: setattr(_ah, "_hook", h)
    _ah.get_axon_ntff_profile_hook = lambda: _ah._hook
    sys.modules["antenv.axon_hooks"] = _ah

B, S, D = 4, 2048, 1024
NB = S // 128          # 16 q-blocks of 128 per batch
NT = S // 512          # 4 q-tiles of 512
IC = D // 128          # 8 contraction chunks
LC = 8                 # local k-chunks per core (S/2/128)
SCALE = 1.0 / np.sqrt(D)  # 0.03125
NJ_TILE = [4, 8, 4, 8]  # local k-chunks needed per gathered q-tile
NWARM = 4              # PE warm-up matmuls

BF16 = mybir.dt.bfloat16
F32 = mybir.dt.float32

_module_cache = None
last_results = None  # BassKernelResults of the most recent run (for test harness)


def _masked_js(tt):
    """Local chunk indices whose score blocks have a boundary block."""
    return range(4) if tt in (0, 2) else range(4, 8)


def _build_module():
    nc = bacc.Bacc("TRN2", target_bir_lowering=False, debug=False, num_devices=8)
    # All inputs are packed partition-major on the host so every input DMA
    # moves multi-KB contiguous runs on both the DRAM and SBUF side.
    # xT is half-major: [h, p, i, c]; m4 is M = Wq^T Wk packed as 4
    # o-group slabs [g, p, i, 256].
    xT = nc.dram_tensor("xT", [2, 128, IC, 512], BF16, kind="ExternalInput").ap()
    m4 = nc.dram_tensor("m4", [4, 128, IC, 256], BF16, kind="ExternalInput").ap()
    wvT = nc.dram_tensor("wvT", [128, IC, 1024], BF16, kind="ExternalInput").ap()
    # msk2[:, 0, :] = shared lower-triangle (own diagonal blocks);
    # msk2[:, 1, :] = all-(1-p) (peer boundary blocks)
    msk2 = nc.dram_tensor("msk2", [128, 2, 128], BF16, kind="ExternalInput").ap()
    out_p = nc.dram_tensor("out_p", [S, D], BF16, kind="ExternalOutput").ap()
    rs_out = nc.dram_tensor("rs_out", [1, S], F32, kind="ExternalOutput").ap()

    with tile.TileContext(nc) as tc:
        with (
            tc.tile_pool(name="wp", bufs=1) as wp,
            tc.tile_pool(name="xp", bufs=1) as xp,
            tc.tile_pool(name="kqv", bufs=1) as kqv,
            tc.tile_pool(name="mp", bufs=1) as mp,
            tc.tile_pool(name="ptp", bufs=2) as ptp,
            tc.tile_pool(name="stg", bufs=4) as stg,
            tc.tile_pool(name="dr", bufs=1, space="DRAM") as dr,
        ):
            # ---- PE warm-up: garbage matmuls on zeroed tiles, issued
            #      before any input-dependent work so the PE leaves its
            #      cold clock state while input DMAs are in flight ----
            warm_w = mp.tile([128, 128], BF16, tag="warmw", name="warmw")
            warm_x = mp.tile([128, 512], BF16, tag="warmx", name="warmx")
            nc.gpsimd.memset(warm_w, 0.0)
            nc.gpsimd.memset(warm_x, 0.0)
            with tc.tile_pool(name="psw", bufs=1, space="PSUM") as psw:
                wpp = psw.tile([128, 512], F32, tag="warm", bufs=1, name="warmp")
                for _ in range(NWARM):
                    nc.tensor.matmul(wpp, lhsT=warm_w, rhs=warm_x, start=True, stop=True)

            xt_all = xp.tile([128, 2, IC, 512], BF16, tag="xt", name="xt")
            m_all = wp.tile([128, 4, IC, 256], BF16, tag="m4", name="m4")
            wv_all = wp.tile([128, IC, 1024], BF16, tag="wv", name="wv")
            msk_sb = mp.tile([128, 2, 128], BF16, tag="msk", name="msk")

            # Queues are FIFO, so order input pieces by when the PE needs
            # them.  sync carries x h0 (then exchange stores, output DMAs);
            # scalar carries everything else.  The G-st0 matmul schedule
            # consumes every slab's i0-3 first, then i4-7, matching this
            # arrival order.  (Two variants measured worse: merging the m4
            # quarters into strided DMAs scatters DRAM reads, and moving
            # x h0 onto scalar in need-order shifts the exchange-accum
            # chain later without improving the ~3.4us ramp stall.)
            nc.sync.dma_start(xt_all[:, 0, 0:4, :], xT[0, :, 0:4, :])
            nc.sync.dma_start(xt_all[:, 0, 4:8, :], xT[0, :, 4:8, :])
            for g in range(4):
                nc.scalar.dma_start(m_all[:, g, 0:4, :], m4[g, :, 0:4, :])
            for g in range(4):
                nc.scalar.dma_start(m_all[:, g, 4:8, :], m4[g, :, 4:8, :])
            nc.scalar.dma_start(xt_all[:, 1, :, :], xT[1])
            nc.scalar.dma_start(wv_all, wvT)
            nc.scalar.dma_start(msk_sb, msk2)
            ones_sb = mp.tile([128, 1], BF16, tag="ones", name="ones")
            nc.any.memset(ones_sb, 1.0)

            # gathered G^T, own-first: cols [0, 1024) own halves st0|st1,
            # cols [1024, 2048) peer halves st0|st1
            gt_all = kqv.tile([128, IC, S], BF16, tag="gt", name="gt")
            vn_sb = [kqv.tile([128, D], BF16, tag=f"vn{j}", name=f"vn{j}") for j in range(LC)]
            rs_sb = mp.tile([1, S], F32, tag="rs", name="rs")

            # AllGather bounce buffers (store src reads gt_all's own
            # region directly -- no SBUF staging copy needed)
            qhalf = [dr.tile([128, IC * 512], BF16, name=f"qhalf{st}") for st in range(2)]
            qfull = [dr.tile([2 * 128, IC * 512], BF16, name=f"qfull{st}") for st in range(2)]

            def xs(i, h):
                return xt_all[:, h, i, :]

            def proj_iouter(ps1, lhs_slices, rhs_slices, dsts, pname, schedule=None, tago=0):
                # schedule: list of (i_range, o_list) sub-groups; flags stay
                # start=(i==0)/stop=(i==IC-1) so any i order that runs i==0
                # first and i==IC-1 last per psum is legal
                schedule = schedule or [(range(IC), range(len(dsts)))]
                pps = [
                    ps1.tile(
                        [128, 512], F32, tag=f"proj8_{o + tago}", bufs=1, name=f"{pname}{o}"
                    )
                    for o in range(len(dsts))
                ]
                for irng, orng in schedule:
                    for i in irng:
                        for o in orng:
                            nc.tensor.matmul(
                                pps[o],
                                lhsT=lhs_slices(i, o),
                                rhs=rhs_slices(i, o),
                                start=(i == 0),
                                stop=(i == IC - 1),
                            )
                for o, dst in enumerate(dsts):
                    dst(pps[o])

            def g_dst(o, st):
                def f(pp):
                    nc.vector.tensor_copy(gt_all[:, o, 512 * st : 512 * (st + 1)], pp)
                    # seed the peer region with -G_own; the two AllGather
                    # rows are then accum-added on top, leaving exactly
                    # G_peer (for the rank whose row equals own, -G + G
                    # cancels exactly)
                    nc.scalar.activation(
                        gt_all[:, o, 1024 + 512 * st : 1024 + 512 * (st + 1)],
                        pp,
                        mybir.ActivationFunctionType.Copy,
                        scale=-1.0,
                    )
                return f

            def g_own_phase(ps1, st, schedule=None):
                """Project this core's own G half straight into gt_all's
                own region, stage the parity-masked RS slots, store, and
                trigger the ReduceScatter."""
                proj_iouter(
                    ps1,
                    lambda i, o: m_all[:, o // 2, i, 128 * (o % 2) : 128 * (o % 2 + 1)],
                    lambda i, o: xs(i, st),
                    [g_dst(o, st) for o in range(IC)],
                    f"pg{st}",
                    schedule=schedule,
                )
                nc.sync.dma_start(
                    qhalf[st].rearrange("p (o c) -> p o c", o=IC),
                    gt_all[:, :, 512 * st : 512 * (st + 1)],
                )
                nc.gpsimd.collective_compute(
                    kind="AllGather",
                    op=mybir.AluOpType.bypass,
                    replica_groups=[[0, 1], [2, 3], [4, 5], [6, 7]],
                    ins=[qhalf[st]],
                    outs=[qfull[st]],
                )

            def peer_accum(st, r):
                # accum-add rank row r of piece st onto the -G_own seed
                # (gpsimd SWDGE is the only DGE that supports accum ops)
                c0 = 1024 + 512 * st
                nc.gpsimd.dma_start(
                    gt_all[:, :, c0 : c0 + 512],
                    qfull[st][128 * r : 128 * (r + 1), :].rearrange(
                        "p (o c) -> p o c", o=IC
                    ),
                    accum_op=mybir.AluOpType.add,
                )

            with tc.tile_pool(name="ps1", bufs=1, space="PSUM") as ps1:
                g_own_phase(
                    ps1,
                    0,
                    schedule=[
                        (range(0, 4), [2 * g, 2 * g + 1]) for g in range(4)
                    ]
                    + [(range(4, 8), [2 * g, 2 * g + 1]) for g in range(4)],
                )
                g_own_phase(ps1, 1)
                peer_accum(0, 0)
                peer_accum(0, 1)
                peer_accum(1, 0)
                peer_accum(1, 1)

                # V projection: all 8 local chunks, four 4-psum rounds
                # with alternating bank tags -- the last round drains
                # banks 4-7, so the score pool's banks (0-3) are free the
                # moment attention tile 0 is ready to start
                for rnd in range(4):
                    proj_iouter(
                        ps1,
                        lambda i, c, _r=rnd: xt_all[
                            :, _r // 2, i, 128 * (2 * (_r % 2) + c // 2) : 128 * (2 * (_r % 2) + c // 2 + 1)
                        ],
                        lambda i, c: wv_all[:, i, 512 * (c % 2) : 512 * (c % 2 + 1)],
                        [
                            (
                                lambda dst, eng: lambda pp: eng(dst, pp)
                            )(
                                vn_sb[4 * (rnd // 2) + 2 * (rnd % 2) + c // 2][
                                    :, 512 * (c % 2) : 512 * (c % 2 + 1)
                                ],
                                nc.vector.tensor_copy if c % 2 == 0 else nc.scalar.copy,
                            )
                            for c in range(4)
                        ],
                        f"pv{rnd}",
                        # (swapping which bank set goes last was measured
                        # neutral: the ~0.9us stall before tile 0's scores
                        # is not the V-copy drain)
                        tago=4 * (rnd % 2),
                    )

            # ---- phase 2: attention -- tiles 0/1 (own half) have no
            #      exchange dependency; tiles 2/3 (peer half) ride the
            #      ReduceScatter outputs ----
            with tc.tile_pool(name="ps2", bufs=2, space="PSUM") as ps:

                def attention_tile(tt):
                    nj = NJ_TILE[tt]
                    masked = set(_masked_js(tt))
                    bsel = 0 if tt < 2 else 1  # triangle vs parity block
                    pt_tiles = []
                    offs = []
                    for j in range(nj):
                        # blocks before the boundary block are fully masked
                        # out -- skip computing them entirely
                        off = 128 * (j % 4) if j in masked else 0
                        offs.append(off)
                        sp = ps.tile([128, 512], F32, tag="score", bufs=4, name="score")
                        for i in range(IC):
                            nc.tensor.matmul(
                                sp[:, off:512],
                                lhsT=xt_all[:, j // 4, i, 128 * (j % 4) : 128 * (j % 4 + 1)],
                                rhs=gt_all[:, i, 512 * tt + off : 512 * (tt + 1)],
                                start=(i == 0),
                                stop=(i == IC - 1),
                            )
                        pt = ptp.tile([128, 512], BF16, tag=f"pt{j}", name=f"pt{j}")
                        nc.scalar.activation(
                            pt[:, off:512],
                            sp[:, off:512],
                            mybir.ActivationFunctionType.Exp,
                            scale=SCALE,
                        )
                        if j in masked:
                            # only the 128-wide boundary block needs the
                            # mask; everything past it is fully visible
                            nc.vector.tensor_mul(
                                pt[:, off : off + 128],
                                pt[:, off : off + 128],
                                msk_sb[:, bsel, :],
                            )
                        pt_tiles.append(pt)

                    # partial softmax denominators: ones^T @ pt over j.
                    # (Folding these into the attn@V loop as 1-column
                    # matmuls costs more than it saves: the 3ns matmuls
                    # leave the next group's LDWEIGHTS exposed, ~94ns x 72.)
                    rsp = ps.tile([1, 512], F32, tag="rsp", bufs=1, name="rsp")
                    for j in range(nj):
                        nc.tensor.matmul(
                            rsp[:, offs[j] : 512],
                            lhsT=ones_sb,
                            rhs=pt_tiles[j][:, offs[j] : 512],
                            start=(j == 0),
                            stop=(j == nj - 1),
                        )
                    nc.vector.tensor_copy(rs_sb[:, 512 * tt : 512 * (tt + 1)], rsp)
                    nc.sync.dma_start(
                        rs_out[:, 512 * tt : 512 * (tt + 1)],
                        rs_sb[:, 512 * tt : 512 * (tt + 1)],
                    )

                    for qq in (3, 2, 1, 0):
                        qbg = 4 * tt + qq        # gathered q-block index
                        njs = (qbg % 8) + 1      # causal chunk count
                        ost = stg.tile([128, D], BF16, tag="ost", name="ost")
                        ap0 = ps.tile([128, 512], F32, tag="attn", bufs=3, name="attn")
                        ap1 = ps.tile([128, 512], F32, tag="attn", bufs=3, name="attn")
                        for j in range(njs):
                            lhs = pt_tiles[j][:, 128 * qq : 128 * (qq + 1)]
                            st_, sp_ = (j == 0), (j == njs - 1)
                            nc.tensor.matmul(
                                ap0, lhsT=lhs, rhs=vn_sb[j][:, 0:512], start=st_, stop=sp_
                            )
                            nc.tensor.matmul(
                                ap1, lhsT=lhs, rhs=vn_sb[j][:, 512:1024], start=st_, stop=sp_
                            )
                        nc.vector.tensor_copy(ost[:, 0:512], ap0)
                        nc.sync.dma_start(
                            out_p[128 * qbg : 128 * (qbg + 1), 0:512], ost[:, 0:512]
                        )
                        nc.scalar.copy(ost[:, 512:1024], ap1)
                        nc.sync.dma_start(
                            out_p[128 * qbg : 128 * (qbg + 1), 512:1024],
                            ost[:, 512:1024],
                        )

                attention_tile(0)
                attention_tile(1)
                attention_tile(2)
                attention_tile(3)

    nc.compile()
    return nc


def _get_module():
    global _module_cache
    if _module_cache is None:
        _module_cache = _build_module()
    return _module_cache


def _glob_map(p):
    """Global q row for each own-first gathered position, parity p."""
    pos = np.arange(S)
    blk = pos // 128
    m = blk % 8
    g = np.where(blk < 8, 2 * m + p, 2 * m + (1 - p))
    return 128 * g + pos % 128


def kernel(x, Wq, Wk, Wv, _trace=False):
    global last_results
    nc = _get_module()

    bf = ml_dtypes.bfloat16

    # M = Wq^T Wk in fp32 (scores = x M x^T), packed as 4 o-group slabs
    # [g, p, i, 256], partition-major
    M = Wq.T.astype(np.float32) @ Wk.astype(np.float32)
    m4 = np.ascontiguousarray(
        M.astype(bf).reshape(IC, 128, 4, 256).transpose(2, 1, 0, 3)
    )
    wvT = np.ascontiguousarray(Wv.T.astype(bf).reshape(IC, 128, D).transpose(1, 0, 2))

    # masks: [:, 0, :] lower-triangle (shared), [:, 1, :] all-(1-p)
    k_i = np.arange(128)[:, None]
    q_i = np.arange(128)[None, :]
    tri = (q_i >= k_i).astype(np.float32)
    msk2 = [
        np.stack([tri, np.full((128, 128), 1.0 - par, dtype=np.float32)], axis=1).astype(bf)
        for par in range(2)
    ]
    # per-parity column selection: core owns global k-chunks {2j+par}
    own_cols = [
        (128 * (2 * np.arange(LC)[:, None] + par) + np.arange(128)[None, :]).reshape(-1)
        for par in range(2)
    ]

    in_maps = []
    for c in range(8):
        b, par = c // 2, c % 2
        xTb = x[b].T[:, own_cols[par]].astype(bf)  # [D, S//2]
        xpk = np.ascontiguousarray(
            xTb.reshape(IC, 128, 2, 512).transpose(2, 1, 0, 3)
        )
        in_maps.append(
            {
                "xT": xpk,
                "m4": m4,
                "wvT": wvT,
                "msk2": np.ascontiguousarray(msk2[par]),
            }
        )

    kwargs = {}
    if _trace:
        kwargs["trace"] = True
    res = run_bass_kernel_spmd(nc, in_maps, core_ids=list(range(8)), **kwargs)
    last_results = res

    gmap = [_glob_map(0), _glob_map(1)]

    out = np.empty((B, S, D), dtype=np.float32)
    for b in range(B):
        num = np.zeros((S, D), dtype=np.float32)
        den = np.zeros(S, dtype=np.float32)
        for par in range(2):
            r = res.results[2 * b + par]
            num[gmap[par]] += r["out_p"].astype(np.float32)
            den[gmap[par]] += r["rs_out"][0]
        out[b] = num / den[:, None]
    return out
